# revision 11
# baseline (speedup 1.0000x reference)
"""Trainium2 Bass kernel for an 8x1024x768 pre-LN transformer encoder block.

Sharding: data-parallel over batch — 8 batch elements -> 8 NeuronCores, no
collectives. Each core runs the full block on its [1024, 768] slice.

Math (per core), reference:
  x = x + Attn(LN1(x));  x = x + FFN(LN2(x))
LN affine (scalar g, b) is folded host-side into the following projection
weights/biases, so the device LN computes (x - mean) / (std_unbiased + eps).

Precision: residual stream, LayerNorm statistics, PSUM accumulation and the
softmax normalization all run in fp32; matmul operands (activations and
weights) are bf16.

Softmax is computed transposed (S^T[k, q]) so no transpose of P is needed:
the denominator comes from the PE "ones-column" trick — V tiles carry a 65th
column of ones, so the P@V matmul also accumulates sum_k P[k, q] in output
row 64. Attention heads are emitted in pairs (partitions 0:64 / 64:128) so
score matmuls of a pair run concurrently in different PE row groups.
"""

import os

import numpy as np
import ml_dtypes

import concourse.bass as bass
import concourse.mybir as mybir
import concourse.tile as tile
from concourse import bacc
from concourse.bass_utils import run_bass_kernel_spmd
from concourse.masks import make_identity

P = 128
NT = 1024          # tokens per core
NI = NT // P       # 8 token chunks
D = 768
KC = D // P        # 6 feature chunks
H = 12
DH = 64
F = 3072
MC = F // P        # 24 ffn chunks
EPS = 1e-5

f32 = mybir.dt.float32
bf16 = mybir.dt.bfloat16

_COMPILE_CACHE = {}
LAST_RESULT = None  # BassKernelResults of the most recent run (for test harness)


def _build(flags):
    has_bqk, has_bv, has_bo, has_b1, has_b2 = flags
    nc = bacc.Bacc("TRN2", target_bir_lowering=False, debug=False, num_devices=8)

    x_d = nc.dram_tensor("x", [NT, D], f32, kind="ExternalInput").ap()
    wq_d = nc.dram_tensor("wq", [D, D], bf16, kind="ExternalInput").ap()
    wk_d = nc.dram_tensor("wk", [D, D], bf16, kind="ExternalInput").ap()
    wv_d = nc.dram_tensor("wv", [D, D], bf16, kind="ExternalInput").ap()
    wo_d = nc.dram_tensor("wo", [D, D], bf16, kind="ExternalInput").ap()
    w1_d = nc.dram_tensor("w1", [D, F], bf16, kind="ExternalInput").ap()
    w2_d = nc.dram_tensor("w2", [F, D], bf16, kind="ExternalInput").ap()
    bq_d = nc.dram_tensor("bq", [D], f32, kind="ExternalInput").ap() if has_bqk else None
    bk_d = nc.dram_tensor("bk", [D], f32, kind="ExternalInput").ap() if has_bqk else None
    bv_d = nc.dram_tensor("bv", [D], f32, kind="ExternalInput").ap() if has_bv else None
    bo_d = nc.dram_tensor("bo", [D], f32, kind="ExternalInput").ap() if has_bo else None
    b1_d = nc.dram_tensor("b1", [F], f32, kind="ExternalInput").ap() if has_b1 else None
    b2_d = nc.dram_tensor("b2", [D], f32, kind="ExternalInput").ap() if has_b2 else None
    out_d = nc.dram_tensor("out", [NT, D], f32, kind="ExternalOutput").ap()

    with tile.TileContext(nc) as tc:
        sb = tc.alloc_tile_pool(name="sb", bufs=1, space="SBUF")
        ps = tc.alloc_tile_pool(name="ps", bufs=1, space="PSUM")

        # ---- constants ----
        def bcast_row(src_ap, n, name):
            # [n] dram vector -> [128, n] sbuf broadcast
            t = sb.tile([P, n], f32, tag=name, bufs=1, name=name)
            nc.sync.dma_start(
                out=t,
                in_=bass.AP(
                    tensor=src_ap.tensor, offset=src_ap.offset, ap=[[0, P], [1, n]]
                ),
            )
            return t

        def chunk_vec(src_ap, nchunk, name):
            # [nchunk*128] dram vector -> [128, nchunk] sbuf (per-partition bias)
            t = sb.tile([P, nchunk], f32, tag=name, bufs=1, name=name)
            nc.sync.dma_start(
                out=t,
                in_=bass.AP(
                    tensor=src_ap.tensor,
                    offset=src_ap.offset,
                    ap=[[1, P], [P, nchunk]],
                ),
            )
            return t

        bq_sb = chunk_vec(bq_d, KC, "bq_sb") if has_bqk else None
        bk_sb = chunk_vec(bk_d, KC, "bk_sb") if has_bqk else None
        b1_sb = chunk_vec(b1_d, MC, "b1_sb") if has_b1 else None
        bv_bc = bcast_row(bv_d, D, "bv_bc") if has_bv else None
        bo_bc = bcast_row(bo_d, D, "bo_bc") if has_bo else None
        b2_bc = bcast_row(b2_d, D, "b2_bc") if has_b2 else None

        # ---- persistent activation tiles ----
        x_t = [sb.tile([P, D], f32, tag="x", bufs=NI, name=f"x{i}") for i in range(NI)]

        def big(name):
            return sb.tile([P, NT], bf16, tag="big", bufs=19, name=name)

        xnT = [big(f"xnT{k}") for k in range(KC)]

        def layernorm(src, dst, i):
            """dst = (src - mean(src)) / (std_unbiased(src) + eps), rowwise."""
            st = sb.tile([P, 3, 6], f32, tag="stat", bufs=2, name=f"st{i}")
            xg = src.rearrange("p (s f) -> p s f", f=256)
            for s in range(3):
                nc.vector.bn_stats(out=st[:, s, :], in_=xg[:, s, :])
            mv = sb.tile([P, 2], f32, tag="mv", bufs=2, name=f"mv{i}")
            nc.vector.bn_aggr(out=mv, in_=st)
            sd = sb.tile([P, 1], f32, tag="sd", bufs=4, name=f"sd{i}")
            # unbiased std = sqrt(var * D/(D-1))
            nc.scalar.activation(
                out=sd, in_=mv[:, 1:2], func=mybir.ActivationFunctionType.Sqrt,
                scale=float(D) / float(D - 1),
            )
            nc.vector.tensor_scalar_add(out=sd, in0=sd, scalar1=EPS)
            rstd = sb.tile([P, 1], f32, tag="sd", bufs=4, name=f"rstd{i}")
            nc.vector.reciprocal(out=rstd, in_=sd)
            nc.vector.tensor_scalar(
                out=dst, in0=src,
                scalar1=mv[:, 0:1], scalar2=rstd,
                op0=mybir.AluOpType.subtract, op1=mybir.AluOpType.mult,
            )

        def transpose_into(xsrc, dstl, i, pfx):
            # xsrc is bf16 [128, 768]; XBAR DMA transpose per 128x128 block
            for k in range(KC):
                nc.sync.dma_start(
                    out=dstl[k][:, i * P : (i + 1) * P],
                    in_=xsrc[:, k * P : (k + 1) * P],
                    transpose=True,
                )

        # ---- V weights + LN1 + transpose + V projection per token chunk ----
        wv_sb = sb.tile([P, KC, D], bf16, tag="wres", bufs=1, name="wv_sb")
        for k in range(KC):
            nc.sync.dma_start(out=wv_sb[:, k, :], in_=wv_d[k * P : (k + 1) * P, :])
        # per head h: cols [65h, 65h+64] = V_h | ones; padded to 848 so a
        # full [128, 128] stationary slab can be loaded for any head.
        v65 = [
            sb.tile([P, 848], bf16, tag="v65", bufs=NI, name=f"v65_{j}")
            for j in range(NI)
        ]
        for i in range(NI):
            nc.gpsimd.dma_start(out=x_t[i], in_=x_d[i * P : (i + 1) * P, :])
            xn = sb.tile([P, D], bf16, tag="xn", bufs=2, name=f"xn{i}")
            layernorm(x_t[i], xn, i)
            transpose_into(xn, xnT, i, "tp")
            j = i
            vaccs = [
                ps.tile([P, 512], f32, tag="smp", bufs=4, name=f"vps{j}_{hf}")
                for hf in range(2)
            ]
            for k in range(KC):
                for hf in range(2):
                    nc.tensor.matmul(
                        vaccs[hf][:, 0:384],
                        xnT[k][:, j * P : (j + 1) * P],
                        wv_sb[:, k, hf * 384 : (hf + 1) * 384],
                        start=(k == 0), stop=(k == KC - 1),
                    )
            for hf in range(2):
                acc = vaccs[hf]
                vview = v65[j][:, hf * 390 : hf * 390 + 390].rearrange(
                    "p (h c) -> p h c", c=DH + 1
                )
                dst = vview[:, :, 0:DH]
                src = acc[:, 0:384].rearrange("p (h c) -> p h c", h=6)
                if has_bv:
                    nc.vector.tensor_add(
                        out=dst, in0=src,
                        in1=bv_bc[:, hf * 384 : (hf + 1) * 384].rearrange(
                            "p (h c) -> p h c", h=6
                        ),
                    )
                else:
                    nc.vector.tensor_copy(out=dst, in_=src)
            nc.vector.memset(
                v65[j][:, 0:780].rearrange("p (h c) -> p h c", c=DH + 1)[:, :, DH:],
                1.0,
            )
            nc.vector.memset(v65[j][:, 780:848], 0.0)

        # ---- Q/K projections (transposed layout [d, tokens]) ----
        qt = [big(f"qt{m}") for m in range(KC)]
        kt = [big(f"kt{m}") for m in range(KC)]
        for (w_d, b_sb, dstl, nm) in ((wq_d, bq_sb, qt, "q"), (wk_d, bk_sb, kt, "k")):
            wslab_d = w_d.rearrange("(kc p) m -> p kc m", p=P)
            for m in range(KC):
                ws = sb.tile([P, KC, P], bf16, tag="wqk", bufs=2, name=f"w{nm}s{m}")
                nc.sync.dma_start(out=ws, in_=wslab_d[:, :, m * P : (m + 1) * P])
                accs = [
                    ps.tile([P, 512], f32, tag="smp", bufs=4, name=f"{nm}ps{m}_{ih}")
                    for ih in range(2)
                ]
                for k in range(KC):
                    for ih in range(2):
                        nc.tensor.matmul(
                            accs[ih],
                            ws[:, k, :],
                            xnT[k][:, ih * 512 : (ih + 1) * 512],
                            start=(k == 0), stop=(k == KC - 1),
                        )
                for ih in range(2):
                    dsl = dstl[m][:, ih * 512 : (ih + 1) * 512]
                    if has_bqk:
                        nc.vector.tensor_scalar_add(
                            out=dsl, in0=accs[ih], scalar1=b_sb[:, m : m + 1]
                        )
                    else:
                        nc.vector.tensor_copy(out=dsl, in_=accs[ih])

        # ---- attention, one head pair at a time ----
        # Heads of a pair sit on partitions 0:64 / 64:128 of qt/kt, so their
        # score matmuls target different PE row groups and run concurrently.
        ot = [big(f"ot{p}") for p in range(KC)]
        for p_ in range(H // 2):
            pt = [[], []]  # per half-pair lists of P^T tiles
            for j in range(NI):
                stp = [
                    ps.tile([P, NT], f32, tag="stp", bufs=2, name=f"st{p_}_{j}_{hh}")
                    for hh in range(2)
                ]
                for hh in range(2):
                    r0, r1 = hh * DH, (hh + 1) * DH
                    for ih in range(2):
                        nc.tensor.matmul(
                            stp[hh][:, ih * 512 : (ih + 1) * 512],
                            kt[p_][r0:r1, j * P : (j + 1) * P],
                            qt[p_][r0:r1, ih * 512 : (ih + 1) * 512],
                            start=True, stop=True,
                        )
                for hh in range(2):
                    ptj = sb.tile(
                        [P, NT], bf16, tag="pt", bufs=18, name=f"pt{p_}_{j}_{hh}"
                    )
                    nc.scalar.activation(
                        out=ptj, in_=stp[hh],
                        func=mybir.ActivationFunctionType.Exp, scale=0.125,
                    )
                    pt[hh].append(ptj)
            for hh in range(2):
                h = 2 * p_ + hh
                r0, r1 = hh * DH, (hh + 1) * DH
                opv = [
                    ps.tile([P, 512], f32, tag="smp", bufs=4, name=f"opv{h}_{iq}")
                    for iq in range(2)
                ]
                for j in range(NI):
                    for iq in range(2):
                        nc.tensor.matmul(
                            opv[iq],
                            v65[j][:, h * (DH + 1) : h * (DH + 1) + P],
                            pt[hh][j][:, iq * 512 : (iq + 1) * 512],
                            start=(j == 0), stop=(j == NI - 1),
                        )
                for iq in range(2):
                    dsb = sb.tile([1, 512], f32, tag="dsb", bufs=2, name=f"dsb{h}_{iq}")
                    nc.vector.tensor_copy(out=dsb, in_=opv[iq][DH : DH + 1, :])
                    rc = sb.tile([1, 512], f32, tag="rc", bufs=2, name=f"rc{h}_{iq}")
                    nc.vector.reciprocal_approx_fast(out=rc, in_=dsb)
                    rb = sb.tile([DH, 512], f32, tag="rb", bufs=2, name=f"rb{h}_{iq}")
                    nc.gpsimd.partition_broadcast(rb, rc)
                    nc.vector.tensor_mul(
                        out=ot[p_][r0:r1, iq * 512 : (iq + 1) * 512],
                        in0=opv[iq][0:DH, :], in1=rb,
                    )

        # ---- O projection + residual (into x_t) ----
        wo_sb = sb.tile([P, KC, D], bf16, tag="wres", bufs=1, name="wo_sb")
        for k in range(KC):
            nc.sync.dma_start(out=wo_sb[:, k, :], in_=wo_d[k * P : (k + 1) * P, :])
        for i in range(NI):
            oaccs = [
                ps.tile([P, 512], f32, tag="smp", bufs=4, name=f"ops{i}_{hf}")
                for hf in range(2)
            ]
            for c in range(KC):
                for hf in range(2):
                    nc.tensor.matmul(
                        oaccs[hf][:, 0:384],
                        ot[c][:, i * P : (i + 1) * P],
                        wo_sb[:, c, hf * 384 : (hf + 1) * 384],
                        start=(c == 0), stop=(c == KC - 1),
                    )
            for hf in range(2):
                xsl = x_t[i][:, hf * 384 : (hf + 1) * 384]
                nc.vector.tensor_add(out=xsl, in0=oaccs[hf][:, 0:384], in1=xsl)
                if has_bo:
                    nc.vector.tensor_add(
                        out=xsl, in0=xsl, in1=bo_bc[:, hf * 384 : (hf + 1) * 384]
                    )

        # ---- LN2 + transpose into xn2T (bf16) ----
        xn2T = [big(f"xn2T{k}") for k in range(KC)]
        for i in range(NI):
            xn2 = sb.tile([P, D], bf16, tag="xn", bufs=2, name=f"xn2_{i}")
            layernorm(x_t[i], xn2, NI + i)
            transpose_into(xn2, xn2T, i, "tq")

        # ---- FFN in 2 token-half passes (streamed w1 slabs + w2 rows) ----
        w1slab_d = w1_d.rearrange("(kc p) m -> p kc m", p=P)
        for ihp in range(2):
            t0 = ihp * 512
            hT = []
            for m in range(MC):
                ws1 = sb.tile([P, KC, P], bf16, tag="w1s", bufs=3, name=f"w1s{ihp}_{m}")
                nc.sync.dma_start(out=ws1, in_=w1slab_d[:, :, m * P : (m + 1) * P])
                acc = ps.tile([P, 512], f32, tag="smp", bufs=4, name=f"fps{ihp}_{m}")
                for k in range(KC):
                    nc.tensor.matmul(
                        acc,
                        ws1[:, k, :],
                        xn2T[k][:, t0 : t0 + 512],
                        start=(k == 0), stop=(k == KC - 1),
                    )
                hTm = sb.tile([P, 512], bf16, tag="hT", bufs=26, name=f"hT{ihp}_{m}")
                nc.scalar.activation(
                    out=hTm, in_=acc,
                    func=mybir.ActivationFunctionType.Gelu,
                    bias=b1_sb[:, m : m + 1] if has_b1 else 0.0,
                )
                hT.append(hTm)
            # FFN2: all 4 token chunks of this half share each streamed w2
            # row-slab; one pass per 384-wide output half. 4 accumulators of
            # [128, 384] live in 2 stp slots at cols [0:384] and [512:896].
            for dh_ in range(2):
                acc2 = [
                    ps.tile([P, NT], f32, tag="stp", bufs=2, name=f"f2ps{ihp}_{dh_}_{sl}")
                    for sl in range(2)
                ]
                accsl = [
                    acc2[il // 2][:, (il % 2) * 512 : (il % 2) * 512 + 384]
                    for il in range(4)
                ]
                w2sl = w2_d[:, dh_ * 384 : (dh_ + 1) * 384].rearrange(
                    "(kc p) n -> p kc n", p=P
                )
                for k2 in range(MC // 2):
                    w2r = sb.tile(
                        [P, 2, 384], bf16, tag="w2r", bufs=3, name=f"w2r{ihp}_{dh_}_{k2}"
                    )
                    nc.sync.dma_start(out=w2r, in_=w2sl[:, 2 * k2 : 2 * k2 + 2, :])
                    for kk in range(2):
                        k = 2 * k2 + kk
                        for il in range(4):
                            nc.tensor.matmul(
                                accsl[il],
                                hT[k][:, il * P : (il + 1) * P],
                                w2r[:, kk, :],
                                start=(k == 0), stop=(k == MC - 1),
                            )
                for il in range(4):
                    i = 4 * ihp + il
                    xsl = x_t[i][:, dh_ * 384 : (dh_ + 1) * 384]
                    nc.vector.tensor_add(out=xsl, in0=accsl[il], in1=xsl)
                    if has_b2:
                        nc.vector.tensor_add(
                            out=xsl, in0=xsl, in1=b2_bc[:, dh_ * 384 : (dh_ + 1) * 384]
                        )
            for il in range(4):
                i = 4 * ihp + il
                nc.gpsimd.dma_start(out=out_d[i * P : (i + 1) * P, :], in_=x_t[i])

        sb.release()
        ps.release()

    nc.compile()
    return nc


def _prep_inputs(inputs):
    """Host-side weight folding. Returns (flags, common_map, per_core_x)."""
    x = np.ascontiguousarray(np.asarray(inputs["x"], dtype=np.float32))
    g1 = float(np.asarray(inputs["g1"]).reshape(-1)[0])
    be1 = float(np.asarray(inputs["be1"]).reshape(-1)[0])
    g2 = float(np.asarray(inputs["g2"]).reshape(-1)[0])
    be2 = float(np.asarray(inputs["be2"]).reshape(-1)[0])

    wq = np.asarray(inputs["wq"], np.float32)
    wk = np.asarray(inputs["wk"], np.float32)
    wv = np.asarray(inputs["wv"], np.float32)
    wo = np.asarray(inputs["wo"], np.float32)
    w1 = np.asarray(inputs["w1"], np.float32)
    w2 = np.asarray(inputs["w2"], np.float32)

    bq = np.asarray(inputs["bq"], np.float32) + be1 * wq.sum(axis=0)
    bk = np.asarray(inputs["bk"], np.float32) + be1 * wk.sum(axis=0)
    bv = np.asarray(inputs["bv"], np.float32) + be1 * wv.sum(axis=0)
    bo = np.asarray(inputs["bo"], np.float32)
    b1 = np.asarray(inputs["b1"], np.float32) + be2 * w1.sum(axis=0)
    b2 = np.asarray(inputs["b2"], np.float32)

    bf = ml_dtypes.bfloat16
    common = {
        "wq": np.ascontiguousarray((g1 * wq).astype(bf)),
        "wk": np.ascontiguousarray((g1 * wk).astype(bf)),
        "wv": np.ascontiguousarray((g1 * wv).astype(bf)),
        "wo": np.ascontiguousarray(wo.astype(bf)),
        "w1": np.ascontiguousarray((g2 * w1).astype(bf)),
        "w2": np.ascontiguousarray(w2.astype(bf)),
    }
    flags = (
        bool(np.any(bq) or np.any(bk)),
        bool(np.any(bv)),
        bool(np.any(bo)),
        bool(np.any(b1)),
        bool(np.any(b2)),
    )
    has_bqk, has_bv, has_bo, has_b1, has_b2 = flags
    if has_bqk:
        common["bq"] = np.ascontiguousarray(bq)
        common["bk"] = np.ascontiguousarray(bk)
    if has_bv:
        common["bv"] = np.ascontiguousarray(bv)
    if has_bo:
        common["bo"] = np.ascontiguousarray(bo)
    if has_b1:
        common["b1"] = np.ascontiguousarray(b1)
    if has_b2:
        common["b2"] = np.ascontiguousarray(b2)
    return flags, common, x


def kernel(**inputs):
    global LAST_RESULT
    flags, common, x = _prep_inputs(inputs)
    if flags not in _COMPILE_CACHE:
        _COMPILE_CACHE[flags] = _build(flags)
    nc = _COMPILE_CACHE[flags]

    n_cores = x.shape[0]
    in_maps = [dict(common, x=np.ascontiguousarray(x[i])) for i in range(n_cores)]
    trace = os.environ.get("BASS_KERNEL_TRACE") == "1"
    res = run_bass_kernel_spmd(nc, in_maps, list(range(n_cores)), trace=trace)
    LAST_RESULT = res
    out = np.stack([res.results[i]["out"] for i in range(n_cores)], axis=0)
    return out.astype(np.float32)


# revision 13
# speedup vs baseline: 1.1344x; 1.1344x over previous
"""Trainium2 Bass kernel for an 8x1024x768 pre-LN transformer encoder block.

Sharding: data-parallel over batch — 8 batch elements -> 8 NeuronCores, no
collectives. Each core runs the full block on its [1024, 768] slice.

Math (per core), reference:
  x = x + Attn(LN1(x));  x = x + FFN(LN2(x))
LN affine (scalar g, b) is folded host-side into the following projection
weights/biases, so the device LN computes (x - mean) / (std_unbiased + eps).

Precision: residual stream, LayerNorm statistics, PSUM accumulation and the
softmax normalization all run in fp32; matmul operands (activations and
weights) are bf16.

Softmax is computed transposed (S^T[k, q]) so no transpose of P is needed:
the denominator comes from the PE "ones-column" trick — V tiles carry a 65th
column of ones, so the P@V matmul also accumulates sum_k P[k, q] in output
row 64. Attention heads are emitted in pairs (partitions 0:64 / 64:128) so
score matmuls of a pair run concurrently in different PE row groups.
"""

import os

import numpy as np
import ml_dtypes

import concourse.bass as bass
import concourse.mybir as mybir
import concourse.tile as tile
from concourse import bacc
from concourse.bass_utils import run_bass_kernel_spmd
from concourse.masks import make_identity

P = 128
NT = 1024          # tokens per core
NI = NT // P       # 8 token chunks
D = 768
KC = D // P        # 6 feature chunks
H = 12
DH = 64
F = 3072
MC = F // P        # 24 ffn chunks
EPS = 1e-5

f32 = mybir.dt.float32
bf16 = mybir.dt.bfloat16

_COMPILE_CACHE = {}
LAST_RESULT = None  # BassKernelResults of the most recent run (for test harness)


def _build(flags):
    has_bqk, has_bv, has_bo, has_b1, has_b2 = flags
    nc = bacc.Bacc("TRN2", target_bir_lowering=False, debug=False, num_devices=8)

    x_d = nc.dram_tensor("x", [NT, D], f32, kind="ExternalInput").ap()
    wq_d = nc.dram_tensor("wq", [D, D], bf16, kind="ExternalInput").ap()
    wk_d = nc.dram_tensor("wk", [D, D], bf16, kind="ExternalInput").ap()
    wv_d = nc.dram_tensor("wv", [D, D], bf16, kind="ExternalInput").ap()
    wo_d = nc.dram_tensor("wo", [D, D], bf16, kind="ExternalInput").ap()
    w1_d = nc.dram_tensor("w1", [D, F], bf16, kind="ExternalInput").ap()
    w2_d = nc.dram_tensor("w2", [F, D], bf16, kind="ExternalInput").ap()
    bq_d = nc.dram_tensor("bq", [D], f32, kind="ExternalInput").ap() if has_bqk else None
    bk_d = nc.dram_tensor("bk", [D], f32, kind="ExternalInput").ap() if has_bqk else None
    bv_d = nc.dram_tensor("bv", [D], f32, kind="ExternalInput").ap() if has_bv else None
    bo_d = nc.dram_tensor("bo", [D], f32, kind="ExternalInput").ap() if has_bo else None
    b1_d = nc.dram_tensor("b1", [F], f32, kind="ExternalInput").ap() if has_b1 else None
    b2_d = nc.dram_tensor("b2", [D], f32, kind="ExternalInput").ap() if has_b2 else None
    out_d = nc.dram_tensor("out", [NT, D], f32, kind="ExternalOutput").ap()

    with tile.TileContext(nc) as tc:
        sb = tc.alloc_tile_pool(name="sb", bufs=1, space="SBUF")
        ps = tc.alloc_tile_pool(name="ps", bufs=1, space="PSUM")

        # ---- constants ----
        ident = sb.tile([P, P], bf16, tag="ident", bufs=1, name="ident")
        make_identity(nc, ident)

        def bcast_row(src_ap, n, name):
            # [n] dram vector -> [128, n] sbuf broadcast
            t = sb.tile([P, n], f32, tag=name, bufs=1, name=name)
            nc.sync.dma_start(
                out=t,
                in_=bass.AP(
                    tensor=src_ap.tensor, offset=src_ap.offset, ap=[[0, P], [1, n]]
                ),
            )
            return t

        def chunk_vec(src_ap, nchunk, name):
            # [nchunk*128] dram vector -> [128, nchunk] sbuf (per-partition bias)
            t = sb.tile([P, nchunk], f32, tag=name, bufs=1, name=name)
            nc.sync.dma_start(
                out=t,
                in_=bass.AP(
                    tensor=src_ap.tensor,
                    offset=src_ap.offset,
                    ap=[[1, P], [P, nchunk]],
                ),
            )
            return t

        bq_sb = chunk_vec(bq_d, KC, "bq_sb") if has_bqk else None
        bk_sb = chunk_vec(bk_d, KC, "bk_sb") if has_bqk else None
        b1_sb = chunk_vec(b1_d, MC, "b1_sb") if has_b1 else None
        bv_bc = bcast_row(bv_d, D, "bv_bc") if has_bv else None
        bo_bc = bcast_row(bo_d, D, "bo_bc") if has_bo else None
        b2_bc = bcast_row(b2_d, D, "b2_bc") if has_b2 else None

        # ---- persistent activation tiles ----
        x_t = [sb.tile([P, D], f32, tag="x", bufs=NI, name=f"x{i}") for i in range(NI)]

        def big(name):
            return sb.tile([P, NT], bf16, tag="big", bufs=19, name=name)

        xnT = [big(f"xnT{k}") for k in range(KC)]

        def layernorm(src, dst, i):
            """dst = (src - mean(src)) / (std_unbiased(src) + eps), rowwise."""
            st = sb.tile([P, 3, 6], f32, tag="stat", bufs=2, name=f"st{i}")
            xg = src.rearrange("p (s f) -> p s f", f=256)
            for s in range(3):
                nc.vector.bn_stats(out=st[:, s, :], in_=xg[:, s, :])
            mv = sb.tile([P, 2], f32, tag="mv", bufs=2, name=f"mv{i}")
            nc.vector.bn_aggr(out=mv, in_=st)
            sd = sb.tile([P, 1], f32, tag="sd", bufs=4, name=f"sd{i}")
            # unbiased std = sqrt(var * D/(D-1))
            nc.scalar.activation(
                out=sd, in_=mv[:, 1:2], func=mybir.ActivationFunctionType.Sqrt,
                scale=float(D) / float(D - 1),
            )
            nc.vector.tensor_scalar_add(out=sd, in0=sd, scalar1=EPS)
            rstd = sb.tile([P, 1], f32, tag="sd", bufs=4, name=f"rstd{i}")
            nc.vector.reciprocal(out=rstd, in_=sd)
            nc.vector.tensor_scalar(
                out=dst, in0=src,
                scalar1=mv[:, 0:1], scalar2=rstd,
                op0=mybir.AluOpType.subtract, op1=mybir.AluOpType.mult,
            )

        def transpose_into(xsrc, dstl, i, pfx):
            # xsrc is bf16 [128, 768]; PE transpose + ACT copy out of PSUM
            for k in range(KC):
                tp = ps.tile([P, 512], bf16, tag="smp", bufs=4, name=f"{pfx}{i}_{k}")
                nc.tensor.transpose(tp[:, 0:P], xsrc[:, k * P : (k + 1) * P], ident)
                nc.scalar.copy(
                    out=dstl[k][:, i * P : (i + 1) * P], in_=tp[:, 0:P]
                )

        # ---- V weights + LN1 + transpose + V projection per token chunk ----
        wv_sb = sb.tile([P, KC, D], bf16, tag="wres", bufs=1, name="wv_sb")
        for k in range(KC):
            nc.sync.dma_start(out=wv_sb[:, k, :], in_=wv_d[k * P : (k + 1) * P, :])
        # per head h: cols [65h, 65h+64] = V_h | ones; padded to 848 so a
        # full [128, 128] stationary slab can be loaded for any head.
        v65 = [
            sb.tile([P, 848], bf16, tag="v65", bufs=NI, name=f"v65_{j}")
            for j in range(NI)
        ]
        for i in range(NI):
            nc.gpsimd.dma_start(out=x_t[i], in_=x_d[i * P : (i + 1) * P, :])
            xn = sb.tile([P, D], bf16, tag="xn", bufs=2, name=f"xn{i}")
            layernorm(x_t[i], xn, i)
            transpose_into(xn, xnT, i, "tp")
            j = i
            vaccs = [
                ps.tile([P, 512], f32, tag="smp", bufs=4, name=f"vps{j}_{hf}")
                for hf in range(2)
            ]
            for k in range(KC):
                for hf in range(2):
                    nc.tensor.matmul(
                        vaccs[hf][:, 0:384],
                        xnT[k][:, j * P : (j + 1) * P],
                        wv_sb[:, k, hf * 384 : (hf + 1) * 384],
                        start=(k == 0), stop=(k == KC - 1),
                    )
            for hf in range(2):
                acc = vaccs[hf]
                vview = v65[j][:, hf * 390 : hf * 390 + 390].rearrange(
                    "p (h c) -> p h c", c=DH + 1
                )
                dst = vview[:, :, 0:DH]
                src = acc[:, 0:384].rearrange("p (h c) -> p h c", h=6)
                if has_bv:
                    nc.vector.tensor_add(
                        out=dst, in0=src,
                        in1=bv_bc[:, hf * 384 : (hf + 1) * 384].rearrange(
                            "p (h c) -> p h c", h=6
                        ),
                    )
                else:
                    nc.vector.tensor_copy(out=dst, in_=src)
            nc.vector.memset(
                v65[j][:, 0:780].rearrange("p (h c) -> p h c", c=DH + 1)[:, :, DH:],
                1.0,
            )
            nc.vector.memset(v65[j][:, 780:848], 0.0)

        # ---- Q/K projections (transposed layout [d, tokens]) ----
        qt = [big(f"qt{m}") for m in range(KC)]
        kt = [big(f"kt{m}") for m in range(KC)]
        for (w_d, b_sb, dstl, nm) in ((wq_d, bq_sb, qt, "q"), (wk_d, bk_sb, kt, "k")):
            wslab_d = w_d.rearrange("(kc p) m -> p kc m", p=P)
            for m in range(KC):
                ws = sb.tile([P, KC, P], bf16, tag="wqk", bufs=2, name=f"w{nm}s{m}")
                nc.sync.dma_start(out=ws, in_=wslab_d[:, :, m * P : (m + 1) * P])
                accs = [
                    ps.tile([P, 512], f32, tag="smp", bufs=4, name=f"{nm}ps{m}_{ih}")
                    for ih in range(2)
                ]
                for k in range(KC):
                    for ih in range(2):
                        nc.tensor.matmul(
                            accs[ih],
                            ws[:, k, :],
                            xnT[k][:, ih * 512 : (ih + 1) * 512],
                            start=(k == 0), stop=(k == KC - 1),
                        )
                for ih in range(2):
                    dsl = dstl[m][:, ih * 512 : (ih + 1) * 512]
                    if has_bqk:
                        nc.vector.tensor_scalar_add(
                            out=dsl, in0=accs[ih], scalar1=b_sb[:, m : m + 1]
                        )
                    else:
                        nc.vector.tensor_copy(out=dsl, in_=accs[ih])

        # ---- attention, one head pair at a time ----
        # Heads of a pair sit on partitions 0:64 / 64:128 of qt/kt, so their
        # score matmuls target different PE row groups and run concurrently.
        ot = [big(f"ot{p}") for p in range(KC)]
        for p_ in range(H // 2):
            pt = [[], []]  # per half-pair lists of P^T tiles
            for j in range(NI):
                stp = [
                    ps.tile([P, NT], f32, tag="stp", bufs=2, name=f"st{p_}_{j}_{hh}")
                    for hh in range(2)
                ]
                for hh in range(2):
                    r0, r1 = hh * DH, (hh + 1) * DH
                    for ih in range(2):
                        nc.tensor.matmul(
                            stp[hh][:, ih * 512 : (ih + 1) * 512],
                            kt[p_][r0:r1, j * P : (j + 1) * P],
                            qt[p_][r0:r1, ih * 512 : (ih + 1) * 512],
                            start=True, stop=True,
                        )
                for hh in range(2):
                    ptj = sb.tile(
                        [P, NT], bf16, tag="pt", bufs=18, name=f"pt{p_}_{j}_{hh}"
                    )
                    nc.scalar.activation(
                        out=ptj, in_=stp[hh],
                        func=mybir.ActivationFunctionType.Exp, scale=0.125,
                    )
                    pt[hh].append(ptj)
            for hh in range(2):
                h = 2 * p_ + hh
                r0, r1 = hh * DH, (hh + 1) * DH
                opv = [
                    ps.tile([P, 512], f32, tag="smp", bufs=4, name=f"opv{h}_{iq}")
                    for iq in range(2)
                ]
                for j in range(NI):
                    for iq in range(2):
                        nc.tensor.matmul(
                            opv[iq],
                            v65[j][:, h * (DH + 1) : h * (DH + 1) + P],
                            pt[hh][j][:, iq * 512 : (iq + 1) * 512],
                            start=(j == 0), stop=(j == NI - 1),
                        )
                for iq in range(2):
                    dsb = sb.tile([1, 512], f32, tag="dsb", bufs=2, name=f"dsb{h}_{iq}")
                    nc.vector.tensor_copy(out=dsb, in_=opv[iq][DH : DH + 1, :])
                    rc = sb.tile([1, 512], f32, tag="rc", bufs=2, name=f"rc{h}_{iq}")
                    nc.vector.reciprocal_approx_fast(out=rc, in_=dsb)
                    rb = sb.tile([DH, 512], f32, tag="rb", bufs=2, name=f"rb{h}_{iq}")
                    nc.gpsimd.partition_broadcast(rb, rc)
                    nc.vector.tensor_mul(
                        out=ot[p_][r0:r1, iq * 512 : (iq + 1) * 512],
                        in0=opv[iq][0:DH, :], in1=rb,
                    )

        # ---- O projection + residual (into x_t) ----
        wo_sb = sb.tile([P, KC, D], bf16, tag="wres", bufs=1, name="wo_sb")
        for k in range(KC):
            nc.sync.dma_start(out=wo_sb[:, k, :], in_=wo_d[k * P : (k + 1) * P, :])
        for i in range(NI):
            oaccs = [
                ps.tile([P, 512], f32, tag="smp", bufs=4, name=f"ops{i}_{hf}")
                for hf in range(2)
            ]
            for c in range(KC):
                for hf in range(2):
                    nc.tensor.matmul(
                        oaccs[hf][:, 0:384],
                        ot[c][:, i * P : (i + 1) * P],
                        wo_sb[:, c, hf * 384 : (hf + 1) * 384],
                        start=(c == 0), stop=(c == KC - 1),
                    )
            for hf in range(2):
                xsl = x_t[i][:, hf * 384 : (hf + 1) * 384]
                nc.vector.tensor_add(out=xsl, in0=oaccs[hf][:, 0:384], in1=xsl)
                if has_bo:
                    nc.vector.tensor_add(
                        out=xsl, in0=xsl, in1=bo_bc[:, hf * 384 : (hf + 1) * 384]
                    )

        # ---- LN2 + transpose into xn2T (bf16) ----
        xn2T = [big(f"xn2T{k}") for k in range(KC)]
        for i in range(NI):
            xn2 = sb.tile([P, D], bf16, tag="xn", bufs=2, name=f"xn2_{i}")
            layernorm(x_t[i], xn2, NI + i)
            transpose_into(xn2, xn2T, i, "tq")

        # ---- FFN in 2 token-half passes (streamed w1 slabs + w2 rows) ----
        w1slab_d = w1_d.rearrange("(kc p) m -> p kc m", p=P)
        for ihp in range(2):
            t0 = ihp * 512
            hT = []
            for m in range(MC):
                ws1 = sb.tile([P, KC, P], bf16, tag="w1s", bufs=3, name=f"w1s{ihp}_{m}")
                nc.sync.dma_start(out=ws1, in_=w1slab_d[:, :, m * P : (m + 1) * P])
                acc = ps.tile([P, 512], f32, tag="smp", bufs=4, name=f"fps{ihp}_{m}")
                for k in range(KC):
                    nc.tensor.matmul(
                        acc,
                        ws1[:, k, :],
                        xn2T[k][:, t0 : t0 + 512],
                        start=(k == 0), stop=(k == KC - 1),
                    )
                hTm = sb.tile([P, 512], bf16, tag="hT", bufs=26, name=f"hT{ihp}_{m}")
                nc.scalar.activation(
                    out=hTm, in_=acc,
                    func=mybir.ActivationFunctionType.Gelu,
                    bias=b1_sb[:, m : m + 1] if has_b1 else 0.0,
                )
                hT.append(hTm)
            # FFN2: all 4 token chunks of this half share each streamed w2
            # row-slab; one pass per 384-wide output half. 4 accumulators of
            # [128, 384] live in 2 stp slots at cols [0:384] and [512:896].
            for dh_ in range(2):
                acc2 = [
                    ps.tile([P, NT], f32, tag="stp", bufs=2, name=f"f2ps{ihp}_{dh_}_{sl}")
                    for sl in range(2)
                ]
                accsl = [
                    acc2[il // 2][:, (il % 2) * 512 : (il % 2) * 512 + 384]
                    for il in range(4)
                ]
                w2sl = w2_d[:, dh_ * 384 : (dh_ + 1) * 384].rearrange(
                    "(kc p) n -> p kc n", p=P
                )
                for k2 in range(MC // 2):
                    w2r = sb.tile(
                        [P, 2, 384], bf16, tag="w2r", bufs=3, name=f"w2r{ihp}_{dh_}_{k2}"
                    )
                    nc.sync.dma_start(out=w2r, in_=w2sl[:, 2 * k2 : 2 * k2 + 2, :])
                    for kk in range(2):
                        k = 2 * k2 + kk
                        for il in range(4):
                            nc.tensor.matmul(
                                accsl[il],
                                hT[k][:, il * P : (il + 1) * P],
                                w2r[:, kk, :],
                                start=(k == 0), stop=(k == MC - 1),
                            )
                for il in range(4):
                    i = 4 * ihp + il
                    xsl = x_t[i][:, dh_ * 384 : (dh_ + 1) * 384]
                    nc.vector.tensor_add(out=xsl, in0=accsl[il], in1=xsl)
                    if has_b2:
                        nc.vector.tensor_add(
                            out=xsl, in0=xsl, in1=b2_bc[:, dh_ * 384 : (dh_ + 1) * 384]
                        )
            for il in range(4):
                i = 4 * ihp + il
                nc.gpsimd.dma_start(out=out_d[i * P : (i + 1) * P, :], in_=x_t[i])

        sb.release()
        ps.release()

    nc.compile()
    return nc


def _prep_inputs(inputs):
    """Host-side weight folding. Returns (flags, common_map, per_core_x)."""
    x = np.ascontiguousarray(np.asarray(inputs["x"], dtype=np.float32))
    g1 = float(np.asarray(inputs["g1"]).reshape(-1)[0])
    be1 = float(np.asarray(inputs["be1"]).reshape(-1)[0])
    g2 = float(np.asarray(inputs["g2"]).reshape(-1)[0])
    be2 = float(np.asarray(inputs["be2"]).reshape(-1)[0])

    wq = np.asarray(inputs["wq"], np.float32)
    wk = np.asarray(inputs["wk"], np.float32)
    wv = np.asarray(inputs["wv"], np.float32)
    wo = np.asarray(inputs["wo"], np.float32)
    w1 = np.asarray(inputs["w1"], np.float32)
    w2 = np.asarray(inputs["w2"], np.float32)

    bq = np.asarray(inputs["bq"], np.float32) + be1 * wq.sum(axis=0)
    bk = np.asarray(inputs["bk"], np.float32) + be1 * wk.sum(axis=0)
    bv = np.asarray(inputs["bv"], np.float32) + be1 * wv.sum(axis=0)
    bo = np.asarray(inputs["bo"], np.float32)
    b1 = np.asarray(inputs["b1"], np.float32) + be2 * w1.sum(axis=0)
    b2 = np.asarray(inputs["b2"], np.float32)

    bf = ml_dtypes.bfloat16
    common = {
        "wq": np.ascontiguousarray((g1 * wq).astype(bf)),
        "wk": np.ascontiguousarray((g1 * wk).astype(bf)),
        "wv": np.ascontiguousarray((g1 * wv).astype(bf)),
        "wo": np.ascontiguousarray(wo.astype(bf)),
        "w1": np.ascontiguousarray((g2 * w1).astype(bf)),
        "w2": np.ascontiguousarray(w2.astype(bf)),
    }
    flags = (
        bool(np.any(bq) or np.any(bk)),
        bool(np.any(bv)),
        bool(np.any(bo)),
        bool(np.any(b1)),
        bool(np.any(b2)),
    )
    has_bqk, has_bv, has_bo, has_b1, has_b2 = flags
    if has_bqk:
        common["bq"] = np.ascontiguousarray(bq)
        common["bk"] = np.ascontiguousarray(bk)
    if has_bv:
        common["bv"] = np.ascontiguousarray(bv)
    if has_bo:
        common["bo"] = np.ascontiguousarray(bo)
    if has_b1:
        common["b1"] = np.ascontiguousarray(b1)
    if has_b2:
        common["b2"] = np.ascontiguousarray(b2)
    return flags, common, x


def kernel(**inputs):
    global LAST_RESULT
    flags, common, x = _prep_inputs(inputs)
    if flags not in _COMPILE_CACHE:
        _COMPILE_CACHE[flags] = _build(flags)
    nc = _COMPILE_CACHE[flags]

    n_cores = x.shape[0]
    in_maps = [dict(common, x=np.ascontiguousarray(x[i])) for i in range(n_cores)]
    trace = os.environ.get("BASS_KERNEL_TRACE") == "1"
    res = run_bass_kernel_spmd(nc, in_maps, list(range(n_cores)), trace=trace)
    LAST_RESULT = res
    out = np.stack([res.results[i]["out"] for i in range(n_cores)], axis=0)
    return out.astype(np.float32)


# revision 14
# speedup vs baseline: 1.1382x; 1.0034x over previous
"""Trainium2 Bass kernel for an 8x1024x768 pre-LN transformer encoder block.

Sharding: data-parallel over batch — 8 batch elements -> 8 NeuronCores, no
collectives. Each core runs the full block on its [1024, 768] slice.

Math (per core), reference:
  x = x + Attn(LN1(x));  x = x + FFN(LN2(x))
LN affine (scalar g, b) is folded host-side into the following projection
weights/biases, so the device LN computes (x - mean) / (std_unbiased + eps).

Precision: residual stream, LayerNorm statistics, PSUM accumulation and the
softmax normalization all run in fp32; matmul operands (activations and
weights) are bf16.

Softmax is computed transposed (S^T[k, q]) so no transpose of P is needed:
the denominator comes from the PE "ones-column" trick — V tiles carry a 65th
column of ones, so the P@V matmul also accumulates sum_k P[k, q] in output
row 64. Attention heads are emitted in pairs (partitions 0:64 / 64:128) so
score matmuls of a pair run concurrently in different PE row groups.
"""

import os

import numpy as np
import ml_dtypes

import concourse.bass as bass
import concourse.mybir as mybir
import concourse.tile as tile
from concourse import bacc
from concourse.bass_utils import run_bass_kernel_spmd
from concourse.masks import make_identity

P = 128
NT = 1024          # tokens per core
NI = NT // P       # 8 token chunks
D = 768
KC = D // P        # 6 feature chunks
H = 12
DH = 64
F = 3072
MC = F // P        # 24 ffn chunks
EPS = 1e-5

f32 = mybir.dt.float32
bf16 = mybir.dt.bfloat16

_COMPILE_CACHE = {}
LAST_RESULT = None  # BassKernelResults of the most recent run (for test harness)


def _build(flags):
    has_bqk, has_bv, has_bo, has_b1, has_b2 = flags
    nc = bacc.Bacc("TRN2", target_bir_lowering=False, debug=False, num_devices=8)

    x_d = nc.dram_tensor("x", [NT, D], f32, kind="ExternalInput").ap()
    wq_d = nc.dram_tensor("wq", [D, D], bf16, kind="ExternalInput").ap()
    wk_d = nc.dram_tensor("wk", [D, D], bf16, kind="ExternalInput").ap()
    wv_d = nc.dram_tensor("wv", [D, D], bf16, kind="ExternalInput").ap()
    wo_d = nc.dram_tensor("wo", [D, D], bf16, kind="ExternalInput").ap()
    w1_d = nc.dram_tensor("w1", [D, F], bf16, kind="ExternalInput").ap()
    w2_d = nc.dram_tensor("w2", [F, D], bf16, kind="ExternalInput").ap()
    bq_d = nc.dram_tensor("bq", [D], f32, kind="ExternalInput").ap() if has_bqk else None
    bk_d = nc.dram_tensor("bk", [D], f32, kind="ExternalInput").ap() if has_bqk else None
    bv_d = nc.dram_tensor("bv", [D], f32, kind="ExternalInput").ap() if has_bv else None
    bo_d = nc.dram_tensor("bo", [D], f32, kind="ExternalInput").ap() if has_bo else None
    b1_d = nc.dram_tensor("b1", [F], f32, kind="ExternalInput").ap() if has_b1 else None
    b2_d = nc.dram_tensor("b2", [D], f32, kind="ExternalInput").ap() if has_b2 else None
    out_d = nc.dram_tensor("out", [NT, D], f32, kind="ExternalOutput").ap()

    with tile.TileContext(nc) as tc:
        sb = tc.alloc_tile_pool(name="sb", bufs=1, space="SBUF")
        ps = tc.alloc_tile_pool(name="ps", bufs=1, space="PSUM")

        # ---- constants ----
        ident = sb.tile([P, P], bf16, tag="ident", bufs=1, name="ident")
        make_identity(nc, ident)

        def bcast_row(src_ap, n, name):
            # [n] dram vector -> [128, n] sbuf broadcast
            t = sb.tile([P, n], f32, tag=name, bufs=1, name=name)
            nc.sync.dma_start(
                out=t,
                in_=bass.AP(
                    tensor=src_ap.tensor, offset=src_ap.offset, ap=[[0, P], [1, n]]
                ),
            )
            return t

        def chunk_vec(src_ap, nchunk, name):
            # [nchunk*128] dram vector -> [128, nchunk] sbuf (per-partition bias)
            t = sb.tile([P, nchunk], f32, tag=name, bufs=1, name=name)
            nc.sync.dma_start(
                out=t,
                in_=bass.AP(
                    tensor=src_ap.tensor,
                    offset=src_ap.offset,
                    ap=[[1, P], [P, nchunk]],
                ),
            )
            return t

        bq_sb = chunk_vec(bq_d, KC, "bq_sb") if has_bqk else None
        bk_sb = chunk_vec(bk_d, KC, "bk_sb") if has_bqk else None
        b1_sb = chunk_vec(b1_d, MC, "b1_sb") if has_b1 else None
        bv_bc = bcast_row(bv_d, D, "bv_bc") if has_bv else None
        bo_bc = bcast_row(bo_d, D, "bo_bc") if has_bo else None
        b2_bc = bcast_row(b2_d, D, "b2_bc") if has_b2 else None

        # ---- persistent activation tiles ----
        x_t = [sb.tile([P, D], f32, tag="x", bufs=NI, name=f"x{i}") for i in range(NI)]

        def big(name):
            return sb.tile([P, NT], bf16, tag="big", bufs=19, name=name)

        xnT = [big(f"xnT{k}") for k in range(KC)]

        def layernorm(src, dst, i):
            """dst = (src - mean(src)) / (std_unbiased(src) + eps), rowwise."""
            st = sb.tile([P, 3, 6], f32, tag="stat", bufs=2, name=f"st{i}")
            xg = src.rearrange("p (s f) -> p s f", f=256)
            for s in range(3):
                nc.vector.bn_stats(out=st[:, s, :], in_=xg[:, s, :])
            mv = sb.tile([P, 2], f32, tag="mv", bufs=2, name=f"mv{i}")
            nc.vector.bn_aggr(out=mv, in_=st)
            sd = sb.tile([P, 1], f32, tag="sd", bufs=4, name=f"sd{i}")
            # unbiased std = sqrt(var * D/(D-1))
            nc.scalar.activation(
                out=sd, in_=mv[:, 1:2], func=mybir.ActivationFunctionType.Sqrt,
                scale=float(D) / float(D - 1),
            )
            nc.vector.tensor_scalar_add(out=sd, in0=sd, scalar1=EPS)
            rstd = sb.tile([P, 1], f32, tag="sd", bufs=4, name=f"rstd{i}")
            nc.vector.reciprocal(out=rstd, in_=sd)
            nc.vector.tensor_scalar(
                out=dst, in0=src,
                scalar1=mv[:, 0:1], scalar2=rstd,
                op0=mybir.AluOpType.subtract, op1=mybir.AluOpType.mult,
            )

        def transpose_into(xsrc, dstl, i, pfx):
            # xsrc is bf16 [128, 768]; PE transpose + ACT copy out of PSUM
            for k in range(KC):
                tp = ps.tile([P, 512], bf16, tag="smp", bufs=2, name=f"{pfx}{i}_{k}")
                nc.tensor.transpose(tp[:, 0:P], xsrc[:, k * P : (k + 1) * P], ident)
                nc.scalar.copy(
                    out=dstl[k][:, i * P : (i + 1) * P], in_=tp[:, 0:P]
                )

        # ---- V weights + LN1 + transpose + V projection per token chunk ----
        wv_sb = sb.tile([P, KC, D], bf16, tag="wres", bufs=1, name="wv_sb")
        for k in range(KC):
            nc.sync.dma_start(out=wv_sb[:, k, :], in_=wv_d[k * P : (k + 1) * P, :])
        # per head h: cols [65h, 65h+64] = V_h | ones; padded to 848 so a
        # full [128, 128] stationary slab can be loaded for any head.
        v65 = [
            sb.tile([P, 848], bf16, tag="v65", bufs=NI, name=f"v65_{j}")
            for j in range(NI)
        ]
        for i in range(NI):
            nc.gpsimd.dma_start(out=x_t[i], in_=x_d[i * P : (i + 1) * P, :])
            xn = sb.tile([P, D], bf16, tag="xn", bufs=2, name=f"xn{i}")
            layernorm(x_t[i], xn, i)
            transpose_into(xn, xnT, i, "tp")
            j = i
            vaccs = [
                ps.tile([P, 512], f32, tag="smp", bufs=2, name=f"vps{j}_{hf}")
                for hf in range(2)
            ]
            for k in range(KC):
                for hf in range(2):
                    nc.tensor.matmul(
                        vaccs[hf][:, 0:384],
                        xnT[k][:, j * P : (j + 1) * P],
                        wv_sb[:, k, hf * 384 : (hf + 1) * 384],
                        start=(k == 0), stop=(k == KC - 1),
                    )
            for hf in range(2):
                acc = vaccs[hf]
                vview = v65[j][:, hf * 390 : hf * 390 + 390].rearrange(
                    "p (h c) -> p h c", c=DH + 1
                )
                dst = vview[:, :, 0:DH]
                src = acc[:, 0:384].rearrange("p (h c) -> p h c", h=6)
                if has_bv:
                    nc.vector.tensor_add(
                        out=dst, in0=src,
                        in1=bv_bc[:, hf * 384 : (hf + 1) * 384].rearrange(
                            "p (h c) -> p h c", h=6
                        ),
                    )
                else:
                    nc.vector.tensor_copy(out=dst, in_=src)
            nc.vector.memset(
                v65[j][:, 0:780].rearrange("p (h c) -> p h c", c=DH + 1)[:, :, DH:],
                1.0,
            )
            nc.vector.memset(v65[j][:, 780:848], 0.0)

        # ---- Q/K projections (transposed layout [d, tokens]) ----
        qt = [big(f"qt{m}") for m in range(KC)]
        kt = [big(f"kt{m}") for m in range(KC)]
        for (w_d, b_sb, dstl, nm) in ((wq_d, bq_sb, qt, "q"), (wk_d, bk_sb, kt, "k")):
            wslab_d = w_d.rearrange("(kc p) m -> p kc m", p=P)
            for m in range(KC):
                ws = sb.tile([P, KC, P], bf16, tag="wqk", bufs=2, name=f"w{nm}s{m}")
                nc.sync.dma_start(out=ws, in_=wslab_d[:, :, m * P : (m + 1) * P])
                accs = [
                    ps.tile([P, 512], f32, tag="smp", bufs=2, name=f"{nm}ps{m}_{ih}")
                    for ih in range(2)
                ]
                for k in range(KC):
                    for ih in range(2):
                        nc.tensor.matmul(
                            accs[ih],
                            ws[:, k, :],
                            xnT[k][:, ih * 512 : (ih + 1) * 512],
                            start=(k == 0), stop=(k == KC - 1),
                        )
                for ih in range(2):
                    dsl = dstl[m][:, ih * 512 : (ih + 1) * 512]
                    if has_bqk:
                        nc.vector.tensor_scalar_add(
                            out=dsl, in0=accs[ih], scalar1=b_sb[:, m : m + 1]
                        )
                    else:
                        nc.vector.tensor_copy(out=dsl, in_=accs[ih])

        # ---- attention, one head pair at a time ----
        # Heads of a pair sit on partitions 0:64 / 64:128 of qt/kt, so their
        # score matmuls target different PE row groups and run concurrently.
        ot = [big(f"ot{p}") for p in range(KC)]
        for h in range(H):
            p_, hh = divmod(h, 2)
            r0, r1 = hh * DH, (hh + 1) * DH
            pt = []
            for j in range(NI):
                stp = ps.tile([P, NT], f32, tag="stp", bufs=3, name=f"st{h}_{j}")
                for ih in range(2):
                    nc.tensor.matmul(
                        stp[:, ih * 512 : (ih + 1) * 512],
                        kt[p_][r0:r1, j * P : (j + 1) * P],
                        qt[p_][r0:r1, ih * 512 : (ih + 1) * 512],
                        start=True, stop=True,
                    )
                ptj = sb.tile([P, NT], bf16, tag="pt", bufs=10, name=f"pt{h}_{j}")
                nc.scalar.activation(
                    out=ptj, in_=stp,
                    func=mybir.ActivationFunctionType.Exp, scale=0.125,
                )
                pt.append(ptj)
            opv = [
                ps.tile([P, 512], f32, tag="smp", bufs=2, name=f"opv{h}_{iq}")
                for iq in range(2)
            ]
            for j in range(NI):
                for iq in range(2):
                    nc.tensor.matmul(
                        opv[iq],
                        v65[j][:, h * (DH + 1) : h * (DH + 1) + P],
                        pt[j][:, iq * 512 : (iq + 1) * 512],
                        start=(j == 0), stop=(j == NI - 1),
                    )
            for iq in range(2):
                dsb = sb.tile([1, 512], f32, tag="dsb", bufs=2, name=f"dsb{h}_{iq}")
                nc.vector.tensor_copy(out=dsb, in_=opv[iq][DH : DH + 1, :])
                rc = sb.tile([1, 512], f32, tag="rc", bufs=2, name=f"rc{h}_{iq}")
                nc.vector.reciprocal_approx_fast(out=rc, in_=dsb)
                rb = sb.tile([DH, 512], f32, tag="rb", bufs=2, name=f"rb{h}_{iq}")
                nc.gpsimd.partition_broadcast(rb, rc)
                nc.vector.tensor_mul(
                    out=ot[p_][r0:r1, iq * 512 : (iq + 1) * 512],
                    in0=opv[iq][0:DH, :], in1=rb,
                )

        # ---- O projection + residual (into x_t) ----
        wo_sb = sb.tile([P, KC, D], bf16, tag="wres", bufs=1, name="wo_sb")
        for k in range(KC):
            nc.sync.dma_start(out=wo_sb[:, k, :], in_=wo_d[k * P : (k + 1) * P, :])
        for i in range(NI):
            oaccs = [
                ps.tile([P, 512], f32, tag="smp", bufs=2, name=f"ops{i}_{hf}")
                for hf in range(2)
            ]
            for c in range(KC):
                for hf in range(2):
                    nc.tensor.matmul(
                        oaccs[hf][:, 0:384],
                        ot[c][:, i * P : (i + 1) * P],
                        wo_sb[:, c, hf * 384 : (hf + 1) * 384],
                        start=(c == 0), stop=(c == KC - 1),
                    )
            for hf in range(2):
                xsl = x_t[i][:, hf * 384 : (hf + 1) * 384]
                nc.vector.tensor_add(out=xsl, in0=oaccs[hf][:, 0:384], in1=xsl)
                if has_bo:
                    nc.vector.tensor_add(
                        out=xsl, in0=xsl, in1=bo_bc[:, hf * 384 : (hf + 1) * 384]
                    )

        # ---- LN2 + transpose into xn2T (bf16) ----
        xn2T = [big(f"xn2T{k}") for k in range(KC)]
        for i in range(NI):
            xn2 = sb.tile([P, D], bf16, tag="xn", bufs=2, name=f"xn2_{i}")
            layernorm(x_t[i], xn2, NI + i)
            transpose_into(xn2, xn2T, i, "tq")

        # ---- FFN in 2 token-half passes (streamed w1 slabs + w2 rows) ----
        w1slab_d = w1_d.rearrange("(kc p) m -> p kc m", p=P)
        for ihp in range(2):
            t0 = ihp * 512
            hT = []
            for m in range(MC):
                ws1 = sb.tile([P, KC, P], bf16, tag="w1s", bufs=3, name=f"w1s{ihp}_{m}")
                nc.sync.dma_start(out=ws1, in_=w1slab_d[:, :, m * P : (m + 1) * P])
                acc = ps.tile([P, 512], f32, tag="smp", bufs=2, name=f"fps{ihp}_{m}")
                for k in range(KC):
                    nc.tensor.matmul(
                        acc,
                        ws1[:, k, :],
                        xn2T[k][:, t0 : t0 + 512],
                        start=(k == 0), stop=(k == KC - 1),
                    )
                hTm = sb.tile([P, 512], bf16, tag="hT", bufs=26, name=f"hT{ihp}_{m}")
                nc.scalar.activation(
                    out=hTm, in_=acc,
                    func=mybir.ActivationFunctionType.Gelu,
                    bias=b1_sb[:, m : m + 1] if has_b1 else 0.0,
                )
                hT.append(hTm)
            # FFN2: all 4 token chunks of this half share each streamed w2
            # row-slab; one pass per 384-wide output half. 4 accumulators of
            # [128, 384] live in 2 stp slots at cols [0:384] and [512:896].
            for dh_ in range(2):
                acc2 = [
                    ps.tile([P, NT], f32, tag="stp", bufs=3, name=f"f2ps{ihp}_{dh_}_{sl}")
                    for sl in range(2)
                ]
                accsl = [
                    acc2[il // 2][:, (il % 2) * 512 : (il % 2) * 512 + 384]
                    for il in range(4)
                ]
                w2sl = w2_d[:, dh_ * 384 : (dh_ + 1) * 384].rearrange(
                    "(kc p) n -> p kc n", p=P
                )
                for k2 in range(MC // 2):
                    w2r = sb.tile(
                        [P, 2, 384], bf16, tag="w2r", bufs=3, name=f"w2r{ihp}_{dh_}_{k2}"
                    )
                    nc.sync.dma_start(out=w2r, in_=w2sl[:, 2 * k2 : 2 * k2 + 2, :])
                    for kk in range(2):
                        k = 2 * k2 + kk
                        for il in range(4):
                            nc.tensor.matmul(
                                accsl[il],
                                hT[k][:, il * P : (il + 1) * P],
                                w2r[:, kk, :],
                                start=(k == 0), stop=(k == MC - 1),
                            )
                for il in range(4):
                    i = 4 * ihp + il
                    xsl = x_t[i][:, dh_ * 384 : (dh_ + 1) * 384]
                    nc.vector.tensor_add(out=xsl, in0=accsl[il], in1=xsl)
                    if has_b2:
                        nc.vector.tensor_add(
                            out=xsl, in0=xsl, in1=b2_bc[:, dh_ * 384 : (dh_ + 1) * 384]
                        )
            for il in range(4):
                i = 4 * ihp + il
                nc.gpsimd.dma_start(out=out_d[i * P : (i + 1) * P, :], in_=x_t[i])

        sb.release()
        ps.release()

    nc.compile()
    return nc


def _prep_inputs(inputs):
    """Host-side weight folding. Returns (flags, common_map, per_core_x)."""
    x = np.ascontiguousarray(np.asarray(inputs["x"], dtype=np.float32))
    g1 = float(np.asarray(inputs["g1"]).reshape(-1)[0])
    be1 = float(np.asarray(inputs["be1"]).reshape(-1)[0])
    g2 = float(np.asarray(inputs["g2"]).reshape(-1)[0])
    be2 = float(np.asarray(inputs["be2"]).reshape(-1)[0])

    wq = np.asarray(inputs["wq"], np.float32)
    wk = np.asarray(inputs["wk"], np.float32)
    wv = np.asarray(inputs["wv"], np.float32)
    wo = np.asarray(inputs["wo"], np.float32)
    w1 = np.asarray(inputs["w1"], np.float32)
    w2 = np.asarray(inputs["w2"], np.float32)

    bq = np.asarray(inputs["bq"], np.float32) + be1 * wq.sum(axis=0)
    bk = np.asarray(inputs["bk"], np.float32) + be1 * wk.sum(axis=0)
    bv = np.asarray(inputs["bv"], np.float32) + be1 * wv.sum(axis=0)
    bo = np.asarray(inputs["bo"], np.float32)
    b1 = np.asarray(inputs["b1"], np.float32) + be2 * w1.sum(axis=0)
    b2 = np.asarray(inputs["b2"], np.float32)

    bf = ml_dtypes.bfloat16
    common = {
        "wq": np.ascontiguousarray((g1 * wq).astype(bf)),
        "wk": np.ascontiguousarray((g1 * wk).astype(bf)),
        "wv": np.ascontiguousarray((g1 * wv).astype(bf)),
        "wo": np.ascontiguousarray(wo.astype(bf)),
        "w1": np.ascontiguousarray((g2 * w1).astype(bf)),
        "w2": np.ascontiguousarray(w2.astype(bf)),
    }
    flags = (
        bool(np.any(bq) or np.any(bk)),
        bool(np.any(bv)),
        bool(np.any(bo)),
        bool(np.any(b1)),
        bool(np.any(b2)),
    )
    has_bqk, has_bv, has_bo, has_b1, has_b2 = flags
    if has_bqk:
        common["bq"] = np.ascontiguousarray(bq)
        common["bk"] = np.ascontiguousarray(bk)
    if has_bv:
        common["bv"] = np.ascontiguousarray(bv)
    if has_bo:
        common["bo"] = np.ascontiguousarray(bo)
    if has_b1:
        common["b1"] = np.ascontiguousarray(b1)
    if has_b2:
        common["b2"] = np.ascontiguousarray(b2)
    return flags, common, x


def kernel(**inputs):
    global LAST_RESULT
    flags, common, x = _prep_inputs(inputs)
    if flags not in _COMPILE_CACHE:
        _COMPILE_CACHE[flags] = _build(flags)
    nc = _COMPILE_CACHE[flags]

    n_cores = x.shape[0]
    in_maps = [dict(common, x=np.ascontiguousarray(x[i])) for i in range(n_cores)]
    trace = os.environ.get("BASS_KERNEL_TRACE") == "1"
    res = run_bass_kernel_spmd(nc, in_maps, list(range(n_cores)), trace=trace)
    LAST_RESULT = res
    out = np.stack([res.results[i]["out"] for i in range(n_cores)], axis=0)
    return out.astype(np.float32)


# revision 15
# speedup vs baseline: 1.1815x; 1.0380x over previous
"""Trainium2 Bass kernel for an 8x1024x768 pre-LN transformer encoder block.

Sharding: data-parallel over batch — 8 batch elements -> 8 NeuronCores, no
collectives. Each core runs the full block on its [1024, 768] slice.

Math (per core), reference:
  x = x + Attn(LN1(x));  x = x + FFN(LN2(x))
LN affine (scalar g, b) is folded host-side into the following projection
weights/biases, so the device LN computes (x - mean) / (std_unbiased + eps).

Precision: residual stream, LayerNorm statistics, PSUM accumulation and the
softmax normalization all run in fp32; matmul operands (activations and
weights) are bf16.

Softmax is computed transposed (S^T[k, q]) so no transpose of P is needed:
the denominator comes from the PE "ones-column" trick — V tiles carry a 65th
column of ones, so the P@V matmul also accumulates sum_k P[k, q] in output
row 64. Attention heads are emitted in pairs (partitions 0:64 / 64:128) so
score matmuls of a pair run concurrently in different PE row groups.
"""

import os

import numpy as np
import ml_dtypes

import concourse.bass as bass
import concourse.mybir as mybir
import concourse.tile as tile
from concourse import bacc
from concourse.bass_utils import run_bass_kernel_spmd
from concourse.masks import make_identity

P = 128
NT = 1024          # tokens per core
NI = NT // P       # 8 token chunks
D = 768
KC = D // P        # 6 feature chunks
H = 12
DH = 64
F = 3072
MC = F // P        # 24 ffn chunks
EPS = 1e-5

f32 = mybir.dt.float32
bf16 = mybir.dt.bfloat16

_COMPILE_CACHE = {}
LAST_RESULT = None  # BassKernelResults of the most recent run (for test harness)


def _build(flags):
    has_bqk, has_bv, has_bo, has_b1, has_b2 = flags
    nc = bacc.Bacc("TRN2", target_bir_lowering=False, debug=False, num_devices=8)

    x_d = nc.dram_tensor("x", [NT, D], f32, kind="ExternalInput").ap()
    wq_d = nc.dram_tensor("wq", [D, D], bf16, kind="ExternalInput").ap()
    wk_d = nc.dram_tensor("wk", [D, D], bf16, kind="ExternalInput").ap()
    wv_d = nc.dram_tensor("wv", [D, D], bf16, kind="ExternalInput").ap()
    wo_d = nc.dram_tensor("wo", [D, D], bf16, kind="ExternalInput").ap()
    w1_d = nc.dram_tensor("w1", [D, F], bf16, kind="ExternalInput").ap()
    w2_d = nc.dram_tensor("w2", [F, D], bf16, kind="ExternalInput").ap()
    bq_d = nc.dram_tensor("bq", [D], f32, kind="ExternalInput").ap() if has_bqk else None
    bk_d = nc.dram_tensor("bk", [D], f32, kind="ExternalInput").ap() if has_bqk else None
    bv_d = nc.dram_tensor("bv", [D], f32, kind="ExternalInput").ap() if has_bv else None
    bo_d = nc.dram_tensor("bo", [D], f32, kind="ExternalInput").ap() if has_bo else None
    b1_d = nc.dram_tensor("b1", [F], f32, kind="ExternalInput").ap() if has_b1 else None
    b2_d = nc.dram_tensor("b2", [D], f32, kind="ExternalInput").ap() if has_b2 else None
    out_d = nc.dram_tensor("out", [NT, D], f32, kind="ExternalOutput").ap()

    with tile.TileContext(nc) as tc:
        sb = tc.alloc_tile_pool(name="sb", bufs=1, space="SBUF")
        ps = tc.alloc_tile_pool(name="ps", bufs=1, space="PSUM")

        # ---- constants ----
        ident = sb.tile([P, P], bf16, tag="ident", bufs=1, name="ident")
        make_identity(nc, ident)

        def bcast_row(src_ap, n, name):
            # [n] dram vector -> [128, n] sbuf broadcast
            t = sb.tile([P, n], f32, tag=name, bufs=1, name=name)
            nc.sync.dma_start(
                out=t,
                in_=bass.AP(
                    tensor=src_ap.tensor, offset=src_ap.offset, ap=[[0, P], [1, n]]
                ),
            )
            return t

        def chunk_vec(src_ap, nchunk, name):
            # [nchunk*128] dram vector -> [128, nchunk] sbuf (per-partition bias)
            t = sb.tile([P, nchunk], f32, tag=name, bufs=1, name=name)
            nc.sync.dma_start(
                out=t,
                in_=bass.AP(
                    tensor=src_ap.tensor,
                    offset=src_ap.offset,
                    ap=[[1, P], [P, nchunk]],
                ),
            )
            return t

        bq_sb = chunk_vec(bq_d, KC, "bq_sb") if has_bqk else None
        bk_sb = chunk_vec(bk_d, KC, "bk_sb") if has_bqk else None
        b1_sb = chunk_vec(b1_d, MC, "b1_sb") if has_b1 else None
        bv_bc = bcast_row(bv_d, D, "bv_bc") if has_bv else None
        bo_bc = bcast_row(bo_d, D, "bo_bc") if has_bo else None
        b2_bc = bcast_row(b2_d, D, "b2_bc") if has_b2 else None

        # ---- persistent activation tiles ----
        x_t = [sb.tile([P, D], f32, tag="x", bufs=NI, name=f"x{i}") for i in range(NI)]

        def big(name):
            return sb.tile([P, NT], bf16, tag="big", bufs=19, name=name)

        xnT = [big(f"xnT{k}") for k in range(KC)]

        def layernorm(src, dst, i):
            """dst = (src - mean(src)) / (std_unbiased(src) + eps), rowwise."""
            st = sb.tile([P, 3, 6], f32, tag="stat", bufs=2, name=f"st{i}")
            xg = src.rearrange("p (s f) -> p s f", f=256)
            for s in range(3):
                nc.vector.bn_stats(out=st[:, s, :], in_=xg[:, s, :])
            mv = sb.tile([P, 2], f32, tag="mv", bufs=2, name=f"mv{i}")
            nc.vector.bn_aggr(out=mv, in_=st)
            sd = sb.tile([P, 1], f32, tag="sd", bufs=4, name=f"sd{i}")
            # unbiased std = sqrt(var * D/(D-1))
            nc.scalar.activation(
                out=sd, in_=mv[:, 1:2], func=mybir.ActivationFunctionType.Sqrt,
                scale=float(D) / float(D - 1),
            )
            nc.vector.tensor_scalar_add(out=sd, in0=sd, scalar1=EPS)
            rstd = sb.tile([P, 1], f32, tag="sd", bufs=4, name=f"rstd{i}")
            nc.vector.reciprocal(out=rstd, in_=sd)
            nc.vector.tensor_scalar(
                out=dst, in0=src,
                scalar1=mv[:, 0:1], scalar2=rstd,
                op0=mybir.AluOpType.subtract, op1=mybir.AluOpType.mult,
            )

        def transpose_into(xsrc, dstl, i, pfx):
            # xsrc is bf16 [128, 768]; PE transpose + ACT copy out of PSUM
            for k in range(KC):
                tp = ps.tile([P, 512], bf16, tag="smp", bufs=2, name=f"{pfx}{i}_{k}")
                nc.tensor.transpose(tp[:, 0:P], xsrc[:, k * P : (k + 1) * P], ident)
                nc.scalar.copy(
                    out=dstl[k][:, i * P : (i + 1) * P], in_=tp[:, 0:P]
                )

        # ---- V weights + LN1 + transpose + V projection per token chunk ----
        wv_sb = sb.tile([P, KC, D], bf16, tag="wres", bufs=1, name="wv_sb")
        for k in range(KC):
            nc.sync.dma_start(out=wv_sb[:, k, :], in_=wv_d[k * P : (k + 1) * P, :])
        # per head h: cols [65h, 65h+64] = V_h | ones; padded to 848 so a
        # full [128, 128] stationary slab can be loaded for any head.
        v65 = [
            sb.tile([P, 848], bf16, tag="v65", bufs=NI, name=f"v65_{j}")
            for j in range(NI)
        ]
        for i in range(NI):
            nc.gpsimd.dma_start(out=x_t[i], in_=x_d[i * P : (i + 1) * P, :])
            xn = sb.tile([P, D], bf16, tag="xn", bufs=2, name=f"xn{i}")
            layernorm(x_t[i], xn, i)
            transpose_into(xn, xnT, i, "tp")
            j = i
            vaccs = [
                ps.tile([P, 512], f32, tag="smp", bufs=2, name=f"vps{j}_{hf}")
                for hf in range(2)
            ]
            for k in range(KC):
                for hf in range(2):
                    nc.tensor.matmul(
                        vaccs[hf][:, 0:384],
                        xnT[k][:, j * P : (j + 1) * P],
                        wv_sb[:, k, hf * 384 : (hf + 1) * 384],
                        start=(k == 0), stop=(k == KC - 1),
                    )
            for hf in range(2):
                acc = vaccs[hf]
                vview = v65[j][:, hf * 390 : hf * 390 + 390].rearrange(
                    "p (h c) -> p h c", c=DH + 1
                )
                dst = vview[:, :, 0:DH]
                src = acc[:, 0:384].rearrange("p (h c) -> p h c", h=6)
                if has_bv:
                    nc.vector.tensor_add(
                        out=dst, in0=src,
                        in1=bv_bc[:, hf * 384 : (hf + 1) * 384].rearrange(
                            "p (h c) -> p h c", h=6
                        ),
                    )
                else:
                    nc.vector.tensor_copy(out=dst, in_=src)
            nc.vector.memset(
                v65[j][:, 0:780].rearrange("p (h c) -> p h c", c=DH + 1)[:, :, DH:],
                1.0,
            )
            nc.vector.memset(v65[j][:, 780:848], 0.0)

        # ---- Q/K projections (transposed layout [d, tokens]) ----
        qt = [big(f"qt{m}") for m in range(KC)]
        kt = [big(f"kt{m}") for m in range(KC)]
        for (w_d, b_sb, dstl, nm) in ((wq_d, bq_sb, qt, "q"), (wk_d, bk_sb, kt, "k")):
            wslab_d = w_d.rearrange("(kc p) m -> p kc m", p=P)
            for m in range(KC):
                ws = sb.tile([P, KC, P], bf16, tag="wqk", bufs=2, name=f"w{nm}s{m}")
                nc.sync.dma_start(out=ws, in_=wslab_d[:, :, m * P : (m + 1) * P])
                accs = [
                    ps.tile([P, 512], f32, tag="smp", bufs=2, name=f"{nm}ps{m}_{ih}")
                    for ih in range(2)
                ]
                for k in range(KC):
                    for ih in range(2):
                        nc.tensor.matmul(
                            accs[ih],
                            ws[:, k, :],
                            xnT[k][:, ih * 512 : (ih + 1) * 512],
                            start=(k == 0), stop=(k == KC - 1),
                        )
                for ih in range(2):
                    dsl = dstl[m][:, ih * 512 : (ih + 1) * 512]
                    if has_bqk:
                        nc.vector.tensor_scalar_add(
                            out=dsl, in0=accs[ih], scalar1=b_sb[:, m : m + 1]
                        )
                    else:
                        nc.vector.tensor_copy(out=dsl, in_=accs[ih])

        # ---- attention, one head pair at a time ----
        # Heads of a pair sit on partitions 0:64 / 64:128 of qt/kt, so their
        # score matmuls target different PE row groups and run concurrently.
        ot = [big(f"ot{p}") for p in range(KC)]
        for h in range(H):
            p_, hh = divmod(h, 2)
            r0, r1 = hh * DH, (hh + 1) * DH
            pt = []
            for j in range(NI):
                stp = ps.tile([P, NT], f32, tag="stp", bufs=3, name=f"st{h}_{j}")
                for ih in range(2):
                    nc.tensor.matmul(
                        stp[:, ih * 512 : (ih + 1) * 512],
                        kt[p_][r0:r1, j * P : (j + 1) * P],
                        qt[p_][r0:r1, ih * 512 : (ih + 1) * 512],
                        start=True, stop=True,
                    )
                ptj = sb.tile([P, NT], bf16, tag="pt", bufs=9, name=f"pt{h}_{j}")
                nc.scalar.activation(
                    out=ptj, in_=stp,
                    func=mybir.ActivationFunctionType.Exp, scale=0.125,
                )
                pt.append(ptj)
            opv = [
                ps.tile([P, 512], f32, tag="smp", bufs=2, name=f"opv{h}_{iq}")
                for iq in range(2)
            ]
            for j in range(NI):
                for iq in range(2):
                    nc.tensor.matmul(
                        opv[iq],
                        v65[j][:, h * (DH + 1) : h * (DH + 1) + P],
                        pt[j][:, iq * 512 : (iq + 1) * 512],
                        start=(j == 0), stop=(j == NI - 1),
                    )
            for iq in range(2):
                dsb = sb.tile([1, 512], f32, tag="dsb", bufs=2, name=f"dsb{h}_{iq}")
                nc.vector.tensor_copy(out=dsb, in_=opv[iq][DH : DH + 1, :])
                rc = sb.tile([1, 512], f32, tag="rc", bufs=2, name=f"rc{h}_{iq}")
                nc.vector.reciprocal_approx_fast(out=rc, in_=dsb)
                rb = sb.tile([DH, 512], f32, tag="rb", bufs=2, name=f"rb{h}_{iq}")
                nc.gpsimd.partition_broadcast(rb, rc)
                nc.vector.tensor_mul(
                    out=ot[p_][r0:r1, iq * 512 : (iq + 1) * 512],
                    in0=opv[iq][0:DH, :], in1=rb,
                )

        # ---- O projection + residual (into x_t) ----
        wo_sb = sb.tile([P, KC, D], bf16, tag="wres", bufs=1, name="wo_sb")
        for k in range(KC):
            nc.sync.dma_start(out=wo_sb[:, k, :], in_=wo_d[k * P : (k + 1) * P, :])
        for i in range(NI):
            oaccs = [
                ps.tile([P, 512], f32, tag="smp", bufs=2, name=f"ops{i}_{hf}")
                for hf in range(2)
            ]
            for c in range(KC):
                for hf in range(2):
                    nc.tensor.matmul(
                        oaccs[hf][:, 0:384],
                        ot[c][:, i * P : (i + 1) * P],
                        wo_sb[:, c, hf * 384 : (hf + 1) * 384],
                        start=(c == 0), stop=(c == KC - 1),
                    )
            for hf in range(2):
                xsl = x_t[i][:, hf * 384 : (hf + 1) * 384]
                nc.vector.tensor_add(out=xsl, in0=oaccs[hf][:, 0:384], in1=xsl)
                if has_bo:
                    nc.vector.tensor_add(
                        out=xsl, in0=xsl, in1=bo_bc[:, hf * 384 : (hf + 1) * 384]
                    )

        # ---- LN2 + transpose into xn2T (bf16) ----
        xn2T = [big(f"xn2T{k}") for k in range(KC)]
        for i in range(NI):
            xn2 = sb.tile([P, D], bf16, tag="xn", bufs=2, name=f"xn2_{i}")
            layernorm(x_t[i], xn2, NI + i)
            transpose_into(xn2, xn2T, i, "tq")

        # ---- FFN1: single pass, full token width, accumulator pair in one
        # stp slot ([0:512] and [512:1024] are different PSUM banks).
        w1slab_d = w1_d.rearrange("(kc p) m -> p kc m", p=P)
        hT = []
        for m in range(MC):
            ws1 = sb.tile([P, KC, P], bf16, tag="w1s", bufs=3, name=f"w1s{m}")
            nc.sync.dma_start(out=ws1, in_=w1slab_d[:, :, m * P : (m + 1) * P])
            acc = ps.tile([P, NT], f32, tag="stp", bufs=3, name=f"fps{m}")
            for k in range(KC):
                for ih in range(2):
                    nc.tensor.matmul(
                        acc[:, ih * 512 : (ih + 1) * 512],
                        ws1[:, k, :],
                        xn2T[k][:, ih * 512 : (ih + 1) * 512],
                        start=(k == 0), stop=(k == KC - 1),
                    )
            hTm = sb.tile([P, NT], bf16, tag="hT", bufs=26, name=f"hT{m}")
            nc.scalar.activation(
                out=hTm, in_=acc,
                func=mybir.ActivationFunctionType.Gelu,
                bias=b1_sb[:, m : m + 1] if has_b1 else 0.0,
            )
            hT.append(hTm)

        # ---- FFN2: per token half, ALL 8 accumulators (4 chunks x 2 d-halves)
        # live across one k-chain: 6 in 3 stp slots + 2 in smp slots. Each
        # w2 row-pair DMA feeds 16 matmuls; each hT stationary feeds 2.
        w2sl = w2_d.rearrange("(kc p) n -> p kc n", p=P)
        for ihp in range(2):
            stps = [
                ps.tile([P, NT], f32, tag="stp", bufs=3, name=f"f2s{ihp}_{sl}")
                for sl in range(3)
            ]
            smps = [
                ps.tile([P, 512], f32, tag="smp", bufs=2, name=f"f2m{ihp}_{sl}")
                for sl in range(2)
            ]
            def accsl(il, dh):
                q = 2 * il + dh
                if q < 6:
                    return stps[q // 2][:, (q % 2) * 512 : (q % 2) * 512 + 384]
                return smps[q - 6][:, 0:384]
            for k2 in range(MC // 2):
                w2r = sb.tile([P, 2, D], bf16, tag="w2r", bufs=3, name=f"w2r{ihp}_{k2}")
                nc.sync.dma_start(out=w2r, in_=w2sl[:, 2 * k2 : 2 * k2 + 2, :])
                for kk in range(2):
                    k = 2 * k2 + kk
                    for il in range(4):
                        for dh_ in range(2):
                            nc.tensor.matmul(
                                accsl(il, dh_),
                                hT[k][:, (4 * ihp + il) * P : (4 * ihp + il + 1) * P],
                                w2r[:, kk, dh_ * 384 : (dh_ + 1) * 384],
                                start=(k == 0), stop=(k == MC - 1),
                            )
            for il in range(4):
                i = 4 * ihp + il
                for dh_ in range(2):
                    xsl = x_t[i][:, dh_ * 384 : (dh_ + 1) * 384]
                    nc.vector.tensor_add(out=xsl, in0=accsl(il, dh_), in1=xsl)
                    if has_b2:
                        nc.vector.tensor_add(
                            out=xsl, in0=xsl, in1=b2_bc[:, dh_ * 384 : (dh_ + 1) * 384]
                        )
                nc.gpsimd.dma_start(out=out_d[i * P : (i + 1) * P, :], in_=x_t[i])

        sb.release()
        ps.release()

    nc.compile()
    return nc


def _prep_inputs(inputs):
    """Host-side weight folding. Returns (flags, common_map, per_core_x)."""
    x = np.ascontiguousarray(np.asarray(inputs["x"], dtype=np.float32))
    g1 = float(np.asarray(inputs["g1"]).reshape(-1)[0])
    be1 = float(np.asarray(inputs["be1"]).reshape(-1)[0])
    g2 = float(np.asarray(inputs["g2"]).reshape(-1)[0])
    be2 = float(np.asarray(inputs["be2"]).reshape(-1)[0])

    wq = np.asarray(inputs["wq"], np.float32)
    wk = np.asarray(inputs["wk"], np.float32)
    wv = np.asarray(inputs["wv"], np.float32)
    wo = np.asarray(inputs["wo"], np.float32)
    w1 = np.asarray(inputs["w1"], np.float32)
    w2 = np.asarray(inputs["w2"], np.float32)

    bq = np.asarray(inputs["bq"], np.float32) + be1 * wq.sum(axis=0)
    bk = np.asarray(inputs["bk"], np.float32) + be1 * wk.sum(axis=0)
    bv = np.asarray(inputs["bv"], np.float32) + be1 * wv.sum(axis=0)
    bo = np.asarray(inputs["bo"], np.float32)
    b1 = np.asarray(inputs["b1"], np.float32) + be2 * w1.sum(axis=0)
    b2 = np.asarray(inputs["b2"], np.float32)

    bf = ml_dtypes.bfloat16
    common = {
        "wq": np.ascontiguousarray((g1 * wq).astype(bf)),
        "wk": np.ascontiguousarray((g1 * wk).astype(bf)),
        "wv": np.ascontiguousarray((g1 * wv).astype(bf)),
        "wo": np.ascontiguousarray(wo.astype(bf)),
        "w1": np.ascontiguousarray((g2 * w1).astype(bf)),
        "w2": np.ascontiguousarray(w2.astype(bf)),
    }
    flags = (
        bool(np.any(bq) or np.any(bk)),
        bool(np.any(bv)),
        bool(np.any(bo)),
        bool(np.any(b1)),
        bool(np.any(b2)),
    )
    has_bqk, has_bv, has_bo, has_b1, has_b2 = flags
    if has_bqk:
        common["bq"] = np.ascontiguousarray(bq)
        common["bk"] = np.ascontiguousarray(bk)
    if has_bv:
        common["bv"] = np.ascontiguousarray(bv)
    if has_bo:
        common["bo"] = np.ascontiguousarray(bo)
    if has_b1:
        common["b1"] = np.ascontiguousarray(b1)
    if has_b2:
        common["b2"] = np.ascontiguousarray(b2)
    return flags, common, x


def kernel(**inputs):
    global LAST_RESULT
    flags, common, x = _prep_inputs(inputs)
    if flags not in _COMPILE_CACHE:
        _COMPILE_CACHE[flags] = _build(flags)
    nc = _COMPILE_CACHE[flags]

    n_cores = x.shape[0]
    in_maps = [dict(common, x=np.ascontiguousarray(x[i])) for i in range(n_cores)]
    trace = os.environ.get("BASS_KERNEL_TRACE") == "1"
    res = run_bass_kernel_spmd(nc, in_maps, list(range(n_cores)), trace=trace)
    LAST_RESULT = res
    out = np.stack([res.results[i]["out"] for i in range(n_cores)], axis=0)
    return out.astype(np.float32)
